# revision 3
# baseline (speedup 1.0000x reference)
"""Bass/Trainium2 kernel for batched GNN message passing:
    out[b, d, n] = sum_m adj[b, n, m] * x[b, d, m]
B=2, D=3072, N=8192, fp32.

Sharding: 8 cores, core c -> (b = c//4, n-quarter = c%4). Each core computes
C[d, n_quarter] = X[b] @ A[b, n_quarter, :].T  with D=3072, NC=2048, M=8192.
Zero collectives; host slices inputs and concatenates outputs.

v2: fp16 operands, host-side transpose (no on-device transposes at all).
Host feeds xT pre-tiled as [kq, db, p, kc, dd] and adjT as plain [M, NC];
per K-pass the kernel streams back-to-back fp16 matmuls (1 cyc/row) into
4 PSUM banks per d-block, alternating bank sets so eviction overlaps
compute. Partial sums across the kq K-passes round-trip DRAM in fp16.
"""

import sys
from contextlib import ExitStack

import numpy as np

sys.path.insert(0, "/opt/trn_rl_repo")

B = 2
D = 3072
N = 8192
NCORES = 8
NSPLIT = 4  # n-quarters per batch sample
NC = N // NSPLIT  # 2048 columns of out per core
KQ = 4  # K-passes
P = 128


def build_program(d=D, ncols=NC, m=N, kq=KQ, nbw=512):
    """Build the per-core Bass program. Returns compiled nc."""
    import concourse.mybir as mybir
    import concourse.tile as tile
    from concourse import bacc

    f32 = mybir.dt.float32
    f16 = mybir.dt.float16

    mq = m // kq          # contraction elems per K-pass
    kc_n = mq // P        # 128-chunks per K-pass
    ndb = d // P          # d-blocks
    nnb = ncols // nbw    # psum banks swept per d-block

    nc = bacc.Bacc(None, target_bir_lowering=False, debug=False)

    # xT tiled on host: x_ext[kqi, db, p, kc, dd] = x[db*P+dd, kqi*mq+kc*P+p]
    x_ext = nc.dram_tensor("x", [kq, ndb, P, kc_n, P], f16, kind="ExternalInput")
    # adjT plain: a_ext[m, n]
    a_ext = nc.dram_tensor("adj", [m, ncols], f16, kind="ExternalInput")
    out_ext = nc.dram_tensor("out", [d, ncols], f32, kind="ExternalOutput")

    with tile.TileContext(nc) as tc, ExitStack() as ctx:
        dram = ctx.enter_context(tc.tile_pool(name="dram", bufs=1, space="DRAM"))
        c_accum = None
        if kq > 1:
            c_accum = dram.tile([ndb, P, ncols], f16, name="c_accum")

        panel_pool = ctx.enter_context(tc.tile_pool(name="panel", bufs=2))
        xt_pool = ctx.enter_context(tc.tile_pool(name="xt", bufs=3))
        stag_pool = ctx.enter_context(tc.tile_pool(name="stag", bufs=3))
        out_pool = ctx.enter_context(tc.tile_pool(name="outp", bufs=2))
        cprev_pool = ctx.enter_context(tc.tile_pool(name="cprev", bufs=3))
        acc_psum = ctx.enter_context(tc.tile_pool(name="accp", bufs=8, space="PSUM"))

        for kqi in range(kq):
            mlo = kqi * mq
            # ---- load adjT panel [P, kc_n, ncols] for this K-pass ----
            panel = panel_pool.tile([P, kc_n, ncols], f16, tag="panel")
            for kc in range(kc_n):
                nc.sync.dma_start(
                    out=panel[:, kc, :],
                    in_=a_ext[mlo + kc * P : mlo + (kc + 1) * P, :],
                )

            # ---- d-block loop ----
            for db in range(ndb):
                xt = xt_pool.tile([P, kc_n, P], f16, tag="xt")
                h = kc_n // 2
                nc.sync.dma_start(out=xt[:, 0:h, :], in_=x_ext[kqi, db, :, 0:h, :])
                nc.sync.dma_start(out=xt[:, h:kc_n, :], in_=x_ext[kqi, db, :, h:kc_n, :])

                cprev = None
                if kqi > 0:
                    cprev = cprev_pool.tile([P, ncols], f16, tag="cprev")
                    hw = ncols // 2
                    nc.sync.dma_start(out=cprev[:, 0:hw], in_=c_accum[db, :, 0:hw])
                    nc.sync.dma_start(
                        out=cprev[:, hw:ncols], in_=c_accum[db, :, hw:ncols]
                    )

                accs = [
                    acc_psum.tile([P, nbw], f32, tag="acc", name=f"acc{i}")
                    for i in range(nnb)
                ]
                for kc in range(kc_n):
                    for nb in range(nnb):
                        nc.tensor.matmul(
                            accs[nb][:],
                            xt[:, kc, :],
                            panel[:, kc, nb * nbw : (nb + 1) * nbw],
                            start=(kc == 0),
                            stop=(kc == kc_n - 1),
                        )

                hw = ncols // 2
                if kqi < kq - 1:
                    stag = stag_pool.tile([P, ncols], f16, tag="stag")
                    for nb in range(nnb):
                        sl = slice(nb * nbw, (nb + 1) * nbw)
                        if kqi == 0:
                            nc.vector.tensor_copy(out=stag[:, sl], in_=accs[nb][:])
                        else:
                            nc.vector.tensor_tensor(
                                out=stag[:, sl],
                                in0=accs[nb][:],
                                in1=cprev[:, sl],
                                op=mybir.AluOpType.add,
                            )
                    nc.sync.dma_start(out=c_accum[db, :, 0:hw], in_=stag[:, 0:hw])
                    nc.sync.dma_start(
                        out=c_accum[db, :, hw:ncols], in_=stag[:, hw:ncols]
                    )
                else:
                    osb = out_pool.tile([P, ncols], f32, tag="osb")
                    for nb in range(nnb):
                        sl = slice(nb * nbw, (nb + 1) * nbw)
                        nc.vector.tensor_tensor(
                            out=osb[:, sl],
                            in0=accs[nb][:],
                            in1=cprev[:, sl],
                            op=mybir.AluOpType.add,
                        )
                    nc.sync.dma_start(
                        out=out_ext[db * P : (db + 1) * P, 0:hw], in_=osb[:, 0:hw]
                    )
                    nc.sync.dma_start(
                        out=out_ext[db * P : (db + 1) * P, hw:ncols],
                        in_=osb[:, hw:ncols],
                    )

    nc.compile()
    return nc


_NC_CACHE = {}


def _get_program(**kw):
    key = tuple(sorted(kw.items()))
    if key not in _NC_CACHE:
        _NC_CACHE[key] = build_program(**kw)
    return _NC_CACHE[key]


def prepare_in_maps(x, adj, kq=KQ):
    """Host-side shard + transpose + fp16 cast. Returns in_maps for 8 cores."""
    f16 = np.float16
    kc_n = (N // kq) // P
    ndb = D // P
    xt_tiled = {}
    for b in range(B):
        xb = x[b].astype(f16)  # [D, M]
        t = xb.reshape(ndb, P, kq, kc_n, P)  # (db, dd, kqi, kc, p)
        xt_tiled[b] = np.ascontiguousarray(t.transpose(2, 0, 4, 3, 1))
    in_maps = []
    for c in range(NCORES):
        b, nq = divmod(c, NSPLIT)
        asl = adj[b, nq * NC : (nq + 1) * NC, :].astype(f16)  # [NC, M]
        in_maps.append({"x": xt_tiled[b], "adj": np.ascontiguousarray(asl.T)})
    return in_maps


def assemble_output(results):
    out = np.empty((B, D, N), dtype=np.float32)
    for c in range(NCORES):
        b, nq = divmod(c, NSPLIT)
        out[b, :, nq * NC : (nq + 1) * NC] = results[c]["out"]
    return out


def kernel(x: np.ndarray, adj: np.ndarray) -> np.ndarray:
    """Full inputs in, full output out. x [B,D,N] f32, adj [B,N,N] f32."""
    from concourse.bass_utils import run_bass_kernel_spmd

    assert x.shape == (B, D, N) and adj.shape == (B, N, N)
    nc = _get_program()
    in_maps = prepare_in_maps(x, adj)
    res = run_bass_kernel_spmd(nc, in_maps, core_ids=list(range(NCORES)))
    return assemble_output(res.results)
